# revision 1
# baseline (speedup 1.0000x reference)
"""Trainium2 Bass kernel for nn_Column_82136954569126 (topk_masking).

Computes: out = einsum('tchw,kchw->tk', rec_field, weight) -> threshold ->
spike stats -> k-WTA top-16 winner mask -> masked spike wave (T, K, 1, 1).

Sharding (8 cores): the contraction C=65536 is split into 8 slices of
8192; every core computes partial sums for ALL 2048 features over its
slice (weight block 64 MiB, rec slice 2.1 MB -> minimal HBM traffic per
core). The contraction is chunked by 128 on the partition dim into
512-wide fp32 accumulating matmuls; even/odd chunks land in the two
PSUM partition halves so each LDWEIGHTS targets the idle column half of
the PE array. The partial (64,2048) is PE-transposed to feature-major
and ReduceScattered across all 8 cores so each core ends up with the
full projection for its own 256 features, derives per-feature ranking scores (a fixed
large bias constant replaces the reference's data-dependent v — the
ranking order is identical), AllGathers the 2048 scores, selects the
global top-16 by rank, and writes its masked spike slice. The host only
re-tiles/shards inputs and re-assembles the output shards. A tiny dummy
collective early in the program absorbs the CC-stream wakeup latency
off the critical tail; DMA triggers alternate between the two HWDGE
engines (Sync/ACT) so descriptor generation pipelines.
"""

import os
import numpy as np

import concourse.bacc as bacc
import concourse.mybir as mybir
import concourse.tile as tile
from concourse import bass_utils

N_CORES = 8
T = 64                 # timesteps
K = 2048               # total output features
P = 128                # SBUF partitions
C = 65536              # full contraction size (1*256*256)
KG = 1                 # k groups
CS = 8                 # contraction split
KW = K // KG           # features per core's matmuls (1024)
NF = KW // 512         # 512-wide matmuls per chunk (2)
KL = KW // CS          # features per core for stats/output (256)
CH = C // CS           # contraction per core (16384)
NCHUNK = CH // P       # contraction chunks per core (128)
THRESH = 16384.0
KWTA = 16
VBIAS = 2097152.0      # constant >> max(n*first_pot); ranking-equivalent to ref's v
WB = 2                 # chunks per weight DMA block (2 MiB)
NWT = NCHUNK // WB     # weight DMA blocks (64)
NSLC = 2               # rec DMA blocks (1 MiB each)
SCH = NCHUNK // NSLC   # chunks per rec block (32)

_nc_cache = None
LAST_RESULT = None


def _build():
    nc = bacc.Bacc("TRN2", target_bir_lowering=False, debug=False,
                   num_devices=N_CORES)
    f32 = mybir.dt.float32

    # Device-tiled layouts (host prepares; every DMA block is contiguous):
    #  rec_dev[s*128+p, ci*T+t] = rec[t, m*CH + (s*SCH+ci)*128 + p]
    #  w_dev[ab*128+p, b*KW+k]  = W[g*KW + k, m*CH + (ab*WB+b)*128 + p]
    rec_in = nc.dram_tensor("rec_dev", [NSLC * P, SCH * T], f32,
                            kind="ExternalInput").ap()
    w_in = nc.dram_tensor("w_dev", [NWT * P, WB * KW], f32,
                          kind="ExternalInput").ap()
    ident_in = nc.dram_tensor("ident", [P, P], f32, kind="ExternalInput").ap()
    iota_in = nc.dram_tensor("iota_t", [1, T], f32, kind="ExternalInput").ap()
    out_spk = nc.dram_tensor("out_spk", [KL, T], f32, kind="ExternalOutput").ap()

    with tile.TileContext(nc) as tc:
        with tc.tile_pool(name="rec", bufs=4) as rec_pool, \
             tc.tile_pool(name="wt", bufs=6) as wt_pool, \
             tc.tile_pool(name="small", bufs=1) as small, \
             tc.tile_pool(name="ps", bufs=1, space="PSUM") as ps, \
             tc.tile_pool(name="pst", bufs=3, space="PSUM") as pst, \
             tc.tile_pool(name="pst1", bufs=1, space="PSUM") as pst1, \
             tc.tile_pool(name="dram", bufs=1, space="DRAM") as dram:

            # starter tiles first in the DMA queue so the PE starts ASAP
            w0a = small.tile([P, KW], f32, tag="w0a")
            nc.sync.dma_start(w0a[:, 0:512], w_in[0:P, 0:512])
            rec0 = small.tile([P, 4 * T], f32, tag="rec0")
            nc.sync.dma_start(rec0[:], rec_in[0:P, 0:4 * T])
            nc.sync.dma_start(w0a[:, 512:KW], w_in[0:P, 512:KW])

            ident = small.tile([P, P], f32)
            nc.sync.dma_start(ident[:], ident_in[:])
            iota_t = small.tile([P, T], f32)
            nc.sync.dma_start(iota_t[:], iota_in.broadcast_to([P, T]))

            # warm up the CC stream early so the first real collective does
            # not pay the ~11 us stream-wakeup on the critical tail
            dum_in = dram.tile([1, 2], f32)
            dum_out = dram.tile([1, 2 * N_CORES], f32)
            nc.sync.dma_start(dum_in[:], ident[0:1, 0:2])
            nc.gpsimd.collective_compute(
                "AllGather", mybir.AluOpType.bypass,
                replica_groups=[list(range(N_CORES))],
                ins=[dum_in.opt()], outs=[dum_out.opt()],
            )

            # ---- the big matmul: acc[t, k] += rec_chunk.T @ w_chunk
            # even chunks -> PSUM partitions 0..63, odd -> 64..127 so each
            # chunk's LDWEIGHTS targets the idle column half of the PE array.
            # chunk -> (rec tile, col offset); first rec slice and first weight
            # block are split small so the PE starts after ~0.5 MB of DMA.
            rec_map = {}
            for i in range(4):
                rec_map[i] = (rec0, i * T)
            rec_loaded = 4

            def load_rec(n_chunks):
                nonlocal rec_loaded
                s = rec_loaded
                r = rec_pool.tile([P, n_chunks * T], f32, tag="recs",
                                  name=f"rec{s}")
                blk, col = divmod(s, SCH)
                nc.sync.dma_start(
                    r[:], rec_in[blk * P:(blk + 1) * P,
                                 col * T:(col + n_chunks) * T])
                for i in range(n_chunks):
                    rec_map[s + i] = (r, i * T)
                rec_loaded += n_chunks

            # two PSUM accumulators (feature halves); within each, even
            # chunks hit partitions 0..63 and odd chunks 64..127 so each
            # LDWEIGHTS targets an idle column half of the PE array.
            accs = [ps.tile([P, 512], f32, name=f"acc{f}") for f in range(NF)]
            grp = NWT // NSLC
            for ab in range(NWT):
                if ab == 0:
                    load_rec(SCH - 4)
                elif ab % grp == grp // 2 and rec_loaded < NCHUNK:
                    load_rec(SCH)
                if ab == 0:
                    w0b = small.tile([P, (WB - 1) * KW], f32, tag="w0b")
                    nc.sync.dma_start(w0b[:], w_in[0:P, KW:WB * KW])
                    w_of = lambda b: (w0a, 0) if b == 0 else \
                        (w0b, (b - 1) * KW)
                else:
                    w_sb = wt_pool.tile([P, WB * KW], f32, tag="w")
                    nc.sync.dma_start(w_sb[:], w_in[ab * P:(ab + 1) * P, :])
                    w_of = lambda b, t=w_sb: (t, b * KW)
                for b in range(WB):
                    a = ab * WB + b
                    r, rof = rec_map[a]
                    wt, wof = w_of(b)
                    hrow = (a & 1) * T
                    for f in range(NF):
                        nc.tensor.matmul(accs[f][hrow:hrow + T, :],
                                         r[:, rof:rof + T],
                                         wt[:, wof + f * 512:wof + (f + 1) * 512],
                                         start=(a < 2), stop=(a >= NCHUNK - 2))

            # ---- combine parity halves, transpose to feature-major [1024, 64]
            # (copies split across ACT and DVE so they run concurrently)
            mm_sb = small.tile([T, KW], f32)
            for f in range(NF):
                cp = nc.vector.tensor_copy if f % 2 == 0 else nc.scalar.copy
                cp(mm_sb[:, f * 512:(f + 1) * 512], accs[f][T:2 * T, :])
            for f in range(NF):
                nc.vector.tensor_tensor(mm_sb[:, f * 512:(f + 1) * 512],
                                        accs[f][0:T, :],
                                        mm_sb[:, f * 512:(f + 1) * 512],
                                        mybir.AluOpType.add)

            # ---- ReduceScatter over the four c-quarters: each core receives
            # the complete projection for its own 256 features.
            rs_in = dram.tile([KW, T], f32)
            rs_out = dram.tile([KL, T], f32)
            outTfull = small.tile([P, (KW // P) * T], f32)
            for q in range(KW // P):
                tq = pst.tile([P, T], f32, tag="tq")
                nc.tensor.transpose(tq[:], mm_sb[:, q * P:(q + 1) * P],
                                    ident[:T, :T])
                cp = nc.vector.tensor_copy if q % 2 == 0 else nc.scalar.copy
                cp(outTfull[:, q * T:(q + 1) * T], tq[:])
                dma = nc.sync.dma_start if q % 2 == 0 else nc.scalar.dma_start
                dma(rs_in[q * P:(q + 1) * P, :],
                    outTfull[:, q * T:(q + 1) * T])
            nc.gpsimd.collective_compute(
                "ReduceScatter", mybir.AluOpType.add,
                replica_groups=[list(range(g * CS, (g + 1) * CS))
                                for g in range(KG)],
                ins=[rs_in.opt()], outs=[rs_out.opt()],
            )
            outT = small.tile([P, 2 * T], f32)   # [k_local(128), half*64 + t]
            for h in range(2):
                dma = nc.sync.dma_start if h == 0 else nc.scalar.dma_start
                dma(outT[:, h * T:(h + 1) * T], rs_out[h * P:(h + 1) * P, :])

            # ---- per-feature stats (k on partitions, t on free dim)
            spikes = small.tile([P, 2 * T], f32)
            score = small.tile([P, 2], f32)
            n_t = small.tile([P, 2], f32)
            scratch = small.tile([P, T], f32)
            for h in range(2):
                ve, sc = nc.vector, scratch
                sl = slice(h * T, (h + 1) * T)
                nh = n_t[:, h:h + 1]
                # spikes = out > thresh, n = sum(spikes)  (fused accumulate)
                ve.tensor_scalar(spikes[:, sl], outT[:, sl], THRESH, 0.0,
                                 mybir.AluOpType.is_gt,
                                 mybir.AluOpType.add, accum_out=nh)
                # first-spike index = T - n ; one-hot match against iota
                fi = small.tile([P, 1], f32, tag=f"fi{h}")
                ve.tensor_scalar(fi[:], nh, -1.0, float(T),
                                 mybir.AluOpType.mult, mybir.AluOpType.add)
                isf = small.tile([P, T], f32, tag=f"isf{h}")
                ve.tensor_scalar(isf[:], iota_t[:, :T], fi[:], None,
                                 mybir.AluOpType.is_equal)
                # one_hot &= spike ; first_pot = sum(out * one_hot)
                ve.scalar_tensor_tensor(isf[:], outT[:, sl], THRESH, isf[:],
                                        mybir.AluOpType.is_gt,
                                        mybir.AluOpType.mult)
                fp = small.tile([P, 1], f32, tag=f"fp{h}")
                ve.scalar_tensor_tensor(sc[:], outT[:, sl], 1.0, isf[:],
                                        mybir.AluOpType.mult,
                                        mybir.AluOpType.mult, accum_out=fp[:])
                # score = (first_pot + VBIAS) * n
                ve.tensor_scalar(score[:, h:h + 1], fp[:], VBIAS, nh,
                                 mybir.AluOpType.add, mybir.AluOpType.mult)

            # ---- AllGather the 256 local scores -> 2048 global scores
            # (pack scores contiguously: transpose [128,2] -> [2,128])
            sT_ps = pst1.tile([2, P], f32, tag="sT")
            nc.tensor.transpose(sT_ps[:], score[:], ident[:])
            sT = small.tile([2, P], f32)
            nc.vector.tensor_copy(sT[:], sT_ps[:])
            s_in = dram.tile([2, P], f32)
            s_out = dram.tile([1, K], f32)
            nc.sync.dma_start(s_in[:], sT[:])
            nc.gpsimd.collective_compute(
                "AllGather", mybir.AluOpType.bypass,
                replica_groups=[list(range(N_CORES))],
                ins=[s_in.opt()], outs=[s_out.opt()],
            )

            # ---- rank each local feature among all 2048 scores
            # (G loaded in halves so ranking pipelines with the broadcast DMA)
            KH = K // 2
            g = small.tile([P, K], f32)
            for q in range(2):
                dma = nc.sync.dma_start if q == 0 else nc.scalar.dma_start
                dma(g[:, q * KH:(q + 1) * KH],
                    s_out[:, q * KH:(q + 1) * KH].broadcast_to([P, KH]))
            masked = small.tile([P, 2 * T], f32)
            cmp = small.tile([P, K], f32)
            rnk = small.tile([P, 4], f32)  # columns: h*2 + half
            ve, cb = nc.vector, cmp
            for h in range(2):
                for q in range(2):
                    col = h * 2 + q
                    ve.tensor_scalar(cb[:, q * KH:(q + 1) * KH],
                                     g[:, q * KH:(q + 1) * KH],
                                     score[:, h:h + 1], 0.0,
                                     mybir.AluOpType.is_gt,
                                     mybir.AluOpType.add,
                                     accum_out=rnk[:, col:col + 1])
            for h in range(2):
                sh = score[:, h:h + 1]
                # rank = #{j : s_all[j] > score_k}
                rank = small.tile([P, 1], f32, tag=f"rank{h}")
                ve.tensor_tensor(rank[:], rnk[:, 2 * h:2 * h + 1],
                                 rnk[:, 2 * h + 1:2 * h + 2],
                                 mybir.AluOpType.add)
                # coef = (rank < KWTA) & (score > 0)
                ltm = small.tile([P, 1], f32, tag=f"ltm{h}")
                ve.tensor_scalar(ltm[:], rank[:], float(KWTA), None,
                                 mybir.AluOpType.is_lt)
                coef = small.tile([P, 1], f32, tag=f"coef{h}")
                ve.scalar_tensor_tensor(coef[:], sh, 0.0, ltm[:],
                                        mybir.AluOpType.is_gt,
                                        mybir.AluOpType.mult)
                sl = slice(h * T, (h + 1) * T)
                ve.tensor_scalar(masked[:, sl], spikes[:, sl], coef[:],
                                 None, mybir.AluOpType.mult)
                nc.sync.dma_start(out_spk[h * P:(h + 1) * P, :], masked[:, sl])

    nc.compile()
    return nc


def kernel(rec_field: np.ndarray, weight: np.ndarray) -> np.ndarray:
    global _nc_cache, LAST_RESULT
    rec = np.ascontiguousarray(rec_field, dtype=np.float32).reshape(T, C)
    w = np.ascontiguousarray(weight, dtype=np.float32).reshape(K, C)

    # host-side re-tiling (sharding layout prep); every DMA block contiguous
    ident = np.eye(P, dtype=np.float32)
    iota_t = np.arange(T, dtype=np.float32)[None, :]

    in_maps = []
    for c in range(N_CORES):
        gk, m = c // CS, c % CS   # RS group = 4 adjacent cores
        rec_m = rec[:, m * CH:(m + 1) * CH]                 # (64, 16384)
        rec_dev = np.ascontiguousarray(
            rec_m.reshape(T, NSLC, SCH, P).transpose(1, 3, 2, 0)
            .reshape(NSLC * P, SCH * T))
        wsh = w[gk * KW:(gk + 1) * KW, m * CH:(m + 1) * CH]  # (1024, 16384)
        w_dev = np.ascontiguousarray(
            wsh.reshape(KW, NWT, WB, P).transpose(1, 3, 2, 0)
            .reshape(NWT * P, WB * KW))
        in_maps.append({
            "rec_dev": rec_dev,
            "w_dev": w_dev,
            "ident": ident,
            "iota_t": iota_t,
        })

    if _nc_cache is None:
        _nc_cache = _build()
    res = bass_utils.run_bass_kernel_spmd(
        _nc_cache, in_maps, core_ids=list(range(N_CORES)),
        trace=bool(os.environ.get("KERNEL_TRACE")),
    )
    LAST_RESULT = res

    full = np.empty((K, T), dtype=np.float32)
    for c in range(N_CORES):
        gk, m = c // CS, c % CS
        k0 = gk * KW + m * KL
        full[k0:k0 + KL] = res.results[c]["out_spk"]
    out = full.T.astype(np.float32)                # (64, 2048)
    return np.ascontiguousarray(out).reshape(T, K, 1, 1)

